# revision 44
# baseline (speedup 1.0000x reference)
"""MoE layer (top-2 routing, 8 experts) on 8 Trainium2 NeuronCores.

Strategy (expert parallelism + 2-way F-sharding for load balance):
  - Host computes the gate (logits -> top-k -> softmax) and routes tokens
    (the host-side equivalent of the all-to-all).
  - Experts are paired (largest token count with smallest); each pair is
    F-sharded across two cores: core 2i holds columns [0, F/2) of experts
    (big_i, small_i), core 2i+1 holds columns [F/2, F).  Each core computes
    partial y sums for ALL tokens of both its experts; the host adds the
    two halves.  This flattens the per-core token count from max_e(count_e)
    to (max big + max small)/2, and keeps all weights SBUF-resident:
      W1 halves 2x2MB + W2 halves 2x2MB per expert -> 16MB/core in SBUF.
  - Per (expert-half, token-chunk) the FFN runs transpose-free:
      mm1:  h^T[f,c] = sum_k W1_blk[k,f].T @ x^T[k,c]   (W1 SBUF-resident)
      gelu: ACT engine, exact (erf) Gelu, bias b1 fused
      mm2:  y^T[d,c] = sum_f W2_blk[f,d].T @ h^T[f,c]   (W2 SBUF-resident)
    Weights/activations bf16 (full PE rate), fp32 PSUM accumulation.
    Weight residency means the PE never waits on weight DMA after the
    first f-tile, so the tensor engine stays at full clock (no HAM
    re-throttle) for the whole kernel.
  - b2 is applied on the host during the combine (y partials exclude it).

Hardcoded problem shape: x [4, 2048, 1024], E=8 experts, D=1024, F=4096.
"""

import numpy as np
import ml_dtypes

import concourse.bass as bass
import concourse.mybir as mybir
import concourse.tile as tile
from concourse import bacc
from concourse.bass_utils import run_bass_kernel_spmd

D = 1024
F = 4096
E = 8
KD = D // 128    # 8 k-tiles over D
FH = F // 2      # F half per shard
KFH = FH // 128  # 16 f-tiles per shard
NT = 512         # max token chunk width (PSUM bank = 512 fp32)

_KERNEL_CACHE = {}


def _chunks(N, first=None):
    """Token chunks (multiples of 8, each <= NT, each >= ~240).

    Any width >= ~240 runs at full PE rate (the ~97ns LDWEIGHTS hides
    behind the previous matmul's streaming); `first` carves off a small
    leading chunk so the kernel head only waits on a small x transfer.
    """
    out, c0 = [], 0
    if first is not None and N >= first + 240:
        out.append((0, first))
        c0 = first
        N -= first
    nch = -(-N // NT)
    base = (N // nch) & ~7
    rem = N - base * nch
    assert rem % 8 == 0
    widths = [base + 8 * (i < rem // 8) for i in range(nch)]
    for w in widths:
        out.append((c0, w))
        c0 += w
    return out


def _build_kernel(N1: int, N2: int):
    """Per-core kernel: two F-half expert shards, weights SBUF-resident.

    Shard A processes N1 tokens, shard B processes N2 tokens (both
    multiples of 8).  Uniform across all 8 cores (SPMD).
    """
    bf16 = mybir.dt.bfloat16
    f32 = mybir.dt.float32

    nc = bacc.Bacc("TRN2", target_bir_lowering=False, debug=False, num_devices=8)

    xd = [
        nc.dram_tensor("xA", [128, KD, N1], bf16, kind="ExternalInput"),
        nc.dram_tensor("xB", [128, KD, N2], bf16, kind="ExternalInput"),
    ]
    w1d = [
        nc.dram_tensor("w1A", [KFH, 128, KD * 128], bf16, kind="ExternalInput"),
        nc.dram_tensor("w1B", [KFH, 128, KD * 128], bf16, kind="ExternalInput"),
    ]
    w2d = [
        nc.dram_tensor("w2A", [KD, 128, KFH * 128], bf16, kind="ExternalInput"),
        nc.dram_tensor("w2B", [KD, 128, KFH * 128], bf16, kind="ExternalInput"),
    ]
    b1d = [
        nc.dram_tensor("b1A", [128, KFH], f32, kind="ExternalInput"),
        nc.dram_tensor("b1B", [128, KFH], f32, kind="ExternalInput"),
    ]
    yd = [
        nc.dram_tensor("yA", [KD, 128, N1], f32, kind="ExternalOutput"),
        nc.dram_tensor("yB", [KD, 128, N2], f32, kind="ExternalOutput"),
    ]
    warm = nc.dram_tensor("warm", [128, 8], f32, kind="ExternalOutput")

    # Flat segment list: (shard s, chunk offset, width).  Segment A leads
    # with a small 256-token chunk whose x comes from a dedicated,
    # per-partition-contiguous tensor: 128 fat descriptors DMA in ~1.4us,
    # against ~7us for the strided 8-per-k path, so the real stream starts
    # ~4us earlier.
    segs = [(0, c0, w) for c0, w in _chunks(N1, first=256)] + [
        (1, c0, w) for c0, w in _chunks(N2)
    ]
    W0 = segs[0][2]
    x0d = nc.dram_tensor("x0A", [128, KD * W0], bf16, kind="ExternalInput")

    with tile.TileContext(nc) as tc:
        with (
            tc.tile_pool(name="const", bufs=1) as const,
            tc.tile_pool(name="w1r", bufs=2 * KFH) as w1r,
            tc.tile_pool(name="w2r", bufs=2 * KD) as w2r,
            tc.tile_pool(name="xp", bufs=3) as xp,
            tc.tile_pool(name="hp", bufs=KFH + 1) as hp,
            tc.tile_pool(name="yp", bufs=4) as yp,
            tc.tile_pool(name="psA", bufs=4, space="PSUM") as psA,
            tc.tile_pool(name="psB", bufs=4, space="PSUM") as psB,
        ):
            # PE warmup: dummy matmuls on zeroed scratch keep the tensor
            # engine busy (ramping to the full HAM p-state) while the first
            # real weight/x DMAs land; sized so the real stream starts fully
            # warm and never stalls after that (a stall costs ~3us of
            # half-clock re-ramp).  The tiny result is stored to a scratch
            # output so nothing gets dead-code-eliminated.  Memsets lead the
            # gpsimd queue so the warmup starts ~6.5us.
            w_s = const.tile([128, 128], bf16, tag="warm_w")
            x_s = const.tile([128, NT], bf16, tag="warm_x")
            nc.gpsimd.memset(w_s[:], 0)
            nc.gpsimd.memset(x_s[:], 0)

            # biases behind the memsets on gpsimd (tiny; needed by first gelu)
            b1_sb = []
            for s in range(2):
                t = const.tile([128, KFH], f32, tag=f"b1_{s}")
                nc.gpsimd.dma_start(t[:], b1d[s][:])
                b1_sb.append(t)
            ps_w = psB.tile([128, NT], f32, tag="ps2")
            NWARM = 12
            for i in range(NWARM):
                nc.tensor.matmul(
                    ps_w[:], w_s[:], x_s[:], start=(i == 0), stop=(i == NWARM - 1)
                )
            warm_sb = const.tile([128, 8], f32, tag="warm_y")
            nc.vector.tensor_scalar_add(warm_sb[:], ps_w[:, :8], 0.0)
            # gpsimd queue is otherwise idle; sync must stay free for weights
            nc.gpsimd.dma_start(warm[:], warm_sb[:])

            # x prefetch ring: one tile per segment chunk, 3 deep, on the
            # scalar queue (HW DGE: setup pipelines with transfers, unlike
            # the gpsimd SWDGE path which costs ~1us serial per dma_start).
            # Chunk 0 instead rides the sync queue, sandwiched between the
            # first two w1A tiles (below).
            x_tiles = {}

            def prefetch_x(si):
                s, c0, w = segs[si]
                t = xp.tile([128, KD, NT], bf16, tag="x")
                nc.scalar.dma_start(t[:, :, :w], xd[s][:, :, c0 : c0 + w])
                x_tiles[si] = t

            prefetch_x(1)

            # resident weights on the sync queue, in consumption order:
            # w1A, w2A, w1B, w2B (per-tile DMAs so the first matmul only
            # waits on its own 256KB slice).  Chunk 0's contiguous x rides
            # this queue after the first two w1A tiles: everything the
            # first ~2us of real work needs lands back-to-back on the
            # earliest-opening HW-DGE queue.
            x0_t = xp.tile([128, KD * W0], bf16, tag="x0")
            w1_sb = [[], []]
            w2_sb = [[], []]
            for s in range(2):
                for fi in range(KFH):
                    t = w1r.tile([128, KD * 128], bf16, tag="w1")
                    nc.sync.dma_start(t[:], w1d[s][fi])
                    w1_sb[s].append(t)
                    if s == 0 and fi == 1:
                        nc.sync.dma_start(x0_t[:], x0d[:])
                for d in range(KD):
                    t = w2r.tile([128, KFH * 128], bf16, tag="w2")
                    nc.sync.dma_start(t[:], w2d[s][d])
                    w2_sb[s].append(t)

            for si, (s, c0, w) in enumerate(segs):
                if si + 2 < len(segs):
                    prefetch_x(si + 2)

                h_t = []
                for fi in range(KFH):
                    ps = psA.tile([128, NT], f32)
                    for k in range(KD):
                        nc.tensor.matmul(
                            ps[:, :w],
                            w1_sb[s][fi][:, k * 128 : (k + 1) * 128],
                            (x0_t[:, k * W0 : k * W0 + w] if si == 0
                             else x_tiles[si][:, k, :w]),
                            start=(k == 0),
                            stop=(k == KD - 1),
                        )
                    ht = hp.tile([128, NT], bf16, tag="h")
                    nc.scalar.activation(
                        ht[:, :w],
                        ps[:, :w],
                        mybir.ActivationFunctionType.Gelu,
                        bias=b1_sb[s][:, fi : fi + 1],
                    )
                    h_t.append(ht)

                for d in range(KD):
                    ps2 = psB.tile([128, NT], f32, tag="ps2")
                    for k2 in range(KFH):
                        nc.tensor.matmul(
                            ps2[:, :w],
                            w2_sb[s][d][:, k2 * 128 : (k2 + 1) * 128],
                            h_t[k2][:, :w],
                            start=(k2 == 0),
                            stop=(k2 == KFH - 1),
                        )
                    y_t = yp.tile([128, NT], f32)
                    nc.vector.tensor_scalar_add(y_t[:, :w], ps2[:, :w], 0.0)
                    # sync queue (HW DGE): the gpsimd SWDGE drain costs ~5us
                    # at kernel tail, the sync drain doesn't
                    nc.sync.dma_start(yd[s][d, :, c0 : c0 + w], y_t[:, :w])

    nc.compile()
    return nc


def _get_kernel(N1: int, N2: int):
    key = (N1, N2)
    if key not in _KERNEL_CACHE:
        _KERNEL_CACHE[key] = _build_kernel(N1, N2)
    return _KERNEL_CACHE[key]


def _route(xf, Wg, bg, top_k):
    """Replicate the reference gate: logits -> top-k -> softmax."""
    logits = xf.astype(np.float32) @ Wg.astype(np.float32) + bg.astype(np.float32)
    # jax.lax.top_k: values sorted descending, ties broken by lower index.
    order = np.argsort(-logits, axis=1, kind="stable")
    sel = order[:, :top_k]                                      # [T, K]
    vals = np.take_along_axis(logits, sel, axis=1)              # [T, K]
    vmax = vals.max(axis=1, keepdims=True)
    ex = np.exp((vals - vmax).astype(np.float32))
    w = ex / ex.sum(axis=1, keepdims=True)                      # [T, K]
    return sel, w.astype(np.float32)


def _plan(x, Wg, bg, top_k):
    """Routing plan: per-expert token indices/gates + big/small pairing."""
    B, S, _ = x.shape
    xf = np.ascontiguousarray(x.reshape(B * S, D).astype(np.float32))
    sel, w = _route(xf, Wg, bg, top_k)
    idx_list, gate_list = [], []
    for e in range(E):
        hit = (sel == e)                    # [T, K]
        tok = np.nonzero(hit.any(axis=1))[0]
        kslot = hit[tok].argmax(axis=1)
        idx_list.append(tok)
        gate_list.append(w[tok, kslot])
    order = sorted(range(E), key=lambda e: -len(idx_list[e]))
    bigs = order[:4]                # 4 largest, descending count
    smalls = order[4:][::-1]        # 4 smallest, ascending count
    pairs = list(zip(bigs, smalls))  # (largest, smallest), ...
    pad = lambda n: max(128, -(-n // 8) * 8)
    N1 = pad(max(len(idx_list[a]) for a, _ in pairs))
    N2 = pad(max(len(idx_list[b]) for _, b in pairs))
    return xf, idx_list, gate_list, pairs, N1, N2


def _pack_x(xf_bf, tok, N):
    xe = np.zeros((N, D), dtype=ml_dtypes.bfloat16)
    xe[: len(tok)] = xf_bf[tok]
    return np.ascontiguousarray(xe.reshape(N, KD, 128).transpose(2, 1, 0))


def _pack_w1(W1e, half):
    sl = W1e[:, half * FH : (half + 1) * FH].astype(ml_dtypes.bfloat16)
    return np.ascontiguousarray(
        sl.reshape(KD, 128, KFH, 128).transpose(2, 1, 0, 3).reshape(KFH, 128, KD * 128)
    )


def _pack_w2(W2e, half):
    sl = W2e[half * FH : (half + 1) * FH].astype(ml_dtypes.bfloat16)
    return np.ascontiguousarray(
        sl.reshape(KFH, 128, KD, 128).transpose(2, 1, 0, 3).reshape(KD, 128, KFH * 128)
    )


def _pack_b1(b1e, half):
    sl = b1e[half * FH : (half + 1) * FH]
    return np.ascontiguousarray(sl.reshape(KFH, 128).T.astype(np.float32))


def _pack_inputs(xf, idx_list, pairs, N1, N2, W1, b1, W2):
    xf_bf = xf.astype(ml_dtypes.bfloat16)
    xA = {}
    xB = {}
    for a, b in pairs:
        xA[a] = _pack_x(xf_bf, idx_list[a], N1)
        xB[b] = _pack_x(xf_bf, idx_list[b], N2)
    W0 = _chunks(N1, first=256)[0][1]
    in_maps = []
    for a, b in pairs:
        x0 = np.ascontiguousarray(xA[a][:, :, :W0]).reshape(128, KD * W0)
        for half in range(2):
            in_maps.append(
                {
                    "xA": xA[a],
                    "x0A": x0,
                    "xB": xB[b],
                    "w1A": _pack_w1(W1[a], half),
                    "w1B": _pack_w1(W1[b], half),
                    "w2A": _pack_w2(W2[a], half),
                    "w2B": _pack_w2(W2[b], half),
                    "b1A": _pack_b1(b1[a], half),
                    "b1B": _pack_b1(b1[b], half),
                }
            )
    return in_maps


def _combine(results, idx_list, gate_list, pairs, N1, N2, T, b2):
    out = np.zeros((T, D), dtype=np.float32)
    for i, (a, b) in enumerate(pairs):
        r0, r1 = results[2 * i], results[2 * i + 1]
        for e, name, N in ((a, "yA", N1), (b, "yB", N2)):
            tok = idx_list[e]
            if len(tok) == 0:
                continue
            y = (r0[name] + r1[name]).transpose(2, 0, 1).reshape(N, D)[: len(tok)]
            g = gate_list[e][:, None]
            out[tok] += g * (y + b2[e][None, :])
    return out


def kernel(x, W1, b1, W2, b2, Wg, bg, top_k):
    x = np.asarray(x)
    W1 = np.asarray(W1, dtype=np.float32)
    b1 = np.asarray(b1, dtype=np.float32)
    W2 = np.asarray(W2, dtype=np.float32)
    b2 = np.asarray(b2, dtype=np.float32)
    Wg = np.asarray(Wg, dtype=np.float32)
    bg = np.asarray(bg, dtype=np.float32)
    top_k = int(np.asarray(top_k))

    B, S, Din = x.shape
    xf, idx_list, gate_list, pairs, N1, N2 = _plan(x, Wg, bg, top_k)
    nc = _get_kernel(N1, N2)
    in_maps = _pack_inputs(xf, idx_list, pairs, N1, N2, W1, b1, W2)
    res = run_bass_kernel_spmd(nc, in_maps, list(range(E)))
    out = _combine(res.results, idx_list, gate_list, pairs, N1, N2, B * S, b2)
    return out.reshape(B, S, Din).astype(np.float32)


# revision 45
# speedup vs baseline: 1.0244x; 1.0244x over previous
"""MoE layer (top-2 routing, 8 experts) on 8 Trainium2 NeuronCores.

Strategy (expert parallelism + 2-way F-sharding for load balance):
  - Host computes the gate (logits -> top-k -> softmax) and routes tokens
    (the host-side equivalent of the all-to-all).
  - Experts are paired (largest token count with smallest); each pair is
    F-sharded across two cores: core 2i holds columns [0, F/2) of experts
    (big_i, small_i), core 2i+1 holds columns [F/2, F).  Each core computes
    partial y sums for ALL tokens of both its experts; the host adds the
    two halves.  This flattens the per-core token count from max_e(count_e)
    to (max big + max small)/2, and keeps all weights SBUF-resident:
      W1 halves 2x2MB + W2 halves 2x2MB per expert -> 16MB/core in SBUF.
  - Per (expert-half, token-chunk) the FFN runs transpose-free:
      mm1:  h^T[f,c] = sum_k W1_blk[k,f].T @ x^T[k,c]   (W1 SBUF-resident)
      gelu: ACT engine, exact (erf) Gelu, bias b1 fused
      mm2:  y^T[d,c] = sum_f W2_blk[f,d].T @ h^T[f,c]   (W2 SBUF-resident)
    Weights/activations bf16 (full PE rate), fp32 PSUM accumulation.
    Weight residency means the PE never waits on weight DMA after the
    first f-tile, so the tensor engine stays at full clock (no HAM
    re-throttle) for the whole kernel.
  - b2 is applied on the host during the combine (y partials exclude it).

Hardcoded problem shape: x [4, 2048, 1024], E=8 experts, D=1024, F=4096.
"""

import numpy as np
import ml_dtypes

import concourse.bass as bass
import concourse.mybir as mybir
import concourse.tile as tile
from concourse import bacc
from concourse.bass_utils import run_bass_kernel_spmd

D = 1024
F = 4096
E = 8
KD = D // 128    # 8 k-tiles over D
FH = F // 2      # F half per shard
KFH = FH // 128  # 16 f-tiles per shard
NT = 512         # max token chunk width (PSUM bank = 512 fp32)

_KERNEL_CACHE = {}


def _chunks(N, first=None):
    """Token chunks (multiples of 8, each <= NT, each >= ~240).

    Any width >= ~240 runs at full PE rate (the ~97ns LDWEIGHTS hides
    behind the previous matmul's streaming); `first` carves off a small
    leading chunk so the kernel head only waits on a small x transfer.
    """
    out, c0 = [], 0
    if first is not None and N >= first + 240:
        out.append((0, first))
        c0 = first
        N -= first
    nch = -(-N // NT)
    base = (N // nch) & ~7
    rem = N - base * nch
    assert rem % 8 == 0
    widths = [base + 8 * (i < rem // 8) for i in range(nch)]
    for w in widths:
        out.append((c0, w))
        c0 += w
    return out


def _build_kernel(N1: int, N2: int):
    """Per-core kernel: two F-half expert shards, weights SBUF-resident.

    Shard A processes N1 tokens, shard B processes N2 tokens (both
    multiples of 8).  Uniform across all 8 cores (SPMD).
    """
    bf16 = mybir.dt.bfloat16
    f32 = mybir.dt.float32

    nc = bacc.Bacc("TRN2", target_bir_lowering=False, debug=False, num_devices=8)

    xd = [
        nc.dram_tensor("xA", [128, KD, N1], bf16, kind="ExternalInput"),
        nc.dram_tensor("xB", [128, KD, N2], bf16, kind="ExternalInput"),
    ]
    w1d = [
        nc.dram_tensor("w1A", [KFH, 128, KD * 128], bf16, kind="ExternalInput"),
        nc.dram_tensor("w1B", [KFH, 128, KD * 128], bf16, kind="ExternalInput"),
    ]
    w2d = [
        nc.dram_tensor("w2A", [KD, 128, KFH * 128], bf16, kind="ExternalInput"),
        nc.dram_tensor("w2B", [KD, 128, KFH * 128], bf16, kind="ExternalInput"),
    ]
    b1d = [
        nc.dram_tensor("b1A", [128, KFH], f32, kind="ExternalInput"),
        nc.dram_tensor("b1B", [128, KFH], f32, kind="ExternalInput"),
    ]
    yd = [
        nc.dram_tensor("yA", [KD, 128, N1], f32, kind="ExternalOutput"),
        nc.dram_tensor("yB", [KD, 128, N2], f32, kind="ExternalOutput"),
    ]
    warm = nc.dram_tensor("warm", [128, 8], f32, kind="ExternalOutput")

    # Flat segment list: (shard s, chunk offset, width)
    segs = [(0, c0, w) for c0, w in _chunks(N1)] + [(1, c0, w) for c0, w in _chunks(N2)]

    with tile.TileContext(nc) as tc:
        with (
            tc.tile_pool(name="const", bufs=1) as const,
            tc.tile_pool(name="w1r", bufs=2 * KFH) as w1r,
            tc.tile_pool(name="w2r", bufs=2 * KD) as w2r,
            tc.tile_pool(name="xp", bufs=3) as xp,
            tc.tile_pool(name="hp", bufs=KFH + 1) as hp,
            tc.tile_pool(name="yp", bufs=4) as yp,
            tc.tile_pool(name="psA", bufs=4, space="PSUM") as psA,
            tc.tile_pool(name="psB", bufs=4, space="PSUM") as psB,
        ):
            # biases first on the gpsimd queue (tiny; needed by first gelu)
            b1_sb = []
            for s in range(2):
                t = const.tile([128, KFH], f32, tag=f"b1_{s}")
                nc.gpsimd.dma_start(t[:], b1d[s][:])
                b1_sb.append(t)

            # PE warmup: dummy matmuls on zeroed scratch keep the tensor
            # engine busy (ramping to the full HAM p-state) while the first
            # real weight/x DMAs land; sized so the real stream starts fully
            # warm and never stalls after that (a stall costs ~3us of
            # half-clock re-ramp).  The tiny result is stored to a scratch
            # output so nothing gets dead-code-eliminated.
            w_s = const.tile([128, 128], bf16, tag="warm_w")
            x_s = const.tile([128, NT], bf16, tag="warm_x")
            nc.gpsimd.memset(w_s[:], 0)
            nc.gpsimd.memset(x_s[:], 0)
            ps_w = psB.tile([128, NT], f32, tag="ps2")
            NWARM = 12
            for i in range(NWARM):
                nc.tensor.matmul(
                    ps_w[:], w_s[:], x_s[:], start=(i == 0), stop=(i == NWARM - 1)
                )
            warm_sb = const.tile([128, 8], f32, tag="warm_y")
            nc.vector.tensor_scalar_add(warm_sb[:], ps_w[:, :8], 0.0)
            # gpsimd queue is otherwise idle; sync must stay free for weights
            nc.gpsimd.dma_start(warm[:], warm_sb[:])

            # x prefetch ring: one tile per segment chunk, 3 deep, on the
            # scalar queue (HW DGE: setup pipelines with transfers, unlike
            # the gpsimd SWDGE path which costs ~1us serial per dma_start).
            x_tiles = {}

            def prefetch_x(si):
                s, c0, w = segs[si]
                t = xp.tile([128, KD, NT], bf16, tag="x")
                if si == 0:
                    # finer-grained arrival for the very first chunk
                    for k in range(KD):
                        nc.scalar.dma_start(t[:, k, :w], xd[s][:, k, c0 : c0 + w])
                else:
                    nc.scalar.dma_start(t[:, :, :w], xd[s][:, :, c0 : c0 + w])
                x_tiles[si] = t

            prefetch_x(0)
            prefetch_x(1)

            # resident weights on the sync queue, in consumption order:
            # w1A, w2A, w1B, w2B (per-tile DMAs so the first matmul only
            # waits on its own 256KB slice)
            w1_sb = [[], []]
            w2_sb = [[], []]
            for s in range(2):
                for fi in range(KFH):
                    t = w1r.tile([128, KD * 128], bf16, tag="w1")
                    nc.sync.dma_start(t[:], w1d[s][fi])
                    w1_sb[s].append(t)
                for d in range(KD):
                    t = w2r.tile([128, KFH * 128], bf16, tag="w2")
                    nc.sync.dma_start(t[:], w2d[s][d])
                    w2_sb[s].append(t)

            for si, (s, c0, w) in enumerate(segs):
                if si + 2 < len(segs):
                    prefetch_x(si + 2)

                h_t = []
                for fi in range(KFH):
                    ps = psA.tile([128, NT], f32)
                    for k in range(KD):
                        nc.tensor.matmul(
                            ps[:, :w],
                            w1_sb[s][fi][:, k * 128 : (k + 1) * 128],
                            x_tiles[si][:, k, :w],
                            start=(k == 0),
                            stop=(k == KD - 1),
                        )
                    ht = hp.tile([128, NT], bf16, tag="h")
                    nc.scalar.activation(
                        ht[:, :w],
                        ps[:, :w],
                        mybir.ActivationFunctionType.Gelu,
                        bias=b1_sb[s][:, fi : fi + 1],
                    )
                    h_t.append(ht)

                for d in range(KD):
                    ps2 = psB.tile([128, NT], f32, tag="ps2")
                    for k2 in range(KFH):
                        nc.tensor.matmul(
                            ps2[:, :w],
                            w2_sb[s][d][:, k2 * 128 : (k2 + 1) * 128],
                            h_t[k2][:, :w],
                            start=(k2 == 0),
                            stop=(k2 == KFH - 1),
                        )
                    y_t = yp.tile([128, NT], f32)
                    nc.vector.tensor_scalar_add(y_t[:, :w], ps2[:, :w], 0.0)
                    # sync queue (HW DGE): the gpsimd SWDGE drain costs ~5us
                    # at kernel tail, the sync drain doesn't
                    nc.sync.dma_start(yd[s][d, :, c0 : c0 + w], y_t[:, :w])

    nc.compile()
    return nc


def _get_kernel(N1: int, N2: int):
    key = (N1, N2)
    if key not in _KERNEL_CACHE:
        _KERNEL_CACHE[key] = _build_kernel(N1, N2)
    return _KERNEL_CACHE[key]


def _route(xf, Wg, bg, top_k):
    """Replicate the reference gate: logits -> top-k -> softmax."""
    logits = xf.astype(np.float32) @ Wg.astype(np.float32) + bg.astype(np.float32)
    # jax.lax.top_k: values sorted descending, ties broken by lower index.
    order = np.argsort(-logits, axis=1, kind="stable")
    sel = order[:, :top_k]                                      # [T, K]
    vals = np.take_along_axis(logits, sel, axis=1)              # [T, K]
    vmax = vals.max(axis=1, keepdims=True)
    ex = np.exp((vals - vmax).astype(np.float32))
    w = ex / ex.sum(axis=1, keepdims=True)                      # [T, K]
    return sel, w.astype(np.float32)


def _plan(x, Wg, bg, top_k):
    """Routing plan: per-expert token indices/gates + big/small pairing."""
    B, S, _ = x.shape
    xf = np.ascontiguousarray(x.reshape(B * S, D).astype(np.float32))
    sel, w = _route(xf, Wg, bg, top_k)
    idx_list, gate_list = [], []
    for e in range(E):
        hit = (sel == e)                    # [T, K]
        tok = np.nonzero(hit.any(axis=1))[0]
        kslot = hit[tok].argmax(axis=1)
        idx_list.append(tok)
        gate_list.append(w[tok, kslot])
    order = sorted(range(E), key=lambda e: -len(idx_list[e]))
    bigs = order[:4]                # 4 largest, descending count
    smalls = order[4:][::-1]        # 4 smallest, ascending count
    pairs = list(zip(bigs, smalls))  # (largest, smallest), ...
    pad = lambda n: max(128, -(-n // 8) * 8)
    N1 = pad(max(len(idx_list[a]) for a, _ in pairs))
    N2 = pad(max(len(idx_list[b]) for _, b in pairs))
    return xf, idx_list, gate_list, pairs, N1, N2


def _pack_x(xf_bf, tok, N):
    xe = np.zeros((N, D), dtype=ml_dtypes.bfloat16)
    xe[: len(tok)] = xf_bf[tok]
    return np.ascontiguousarray(xe.reshape(N, KD, 128).transpose(2, 1, 0))


def _pack_w1(W1e, half):
    sl = W1e[:, half * FH : (half + 1) * FH].astype(ml_dtypes.bfloat16)
    return np.ascontiguousarray(
        sl.reshape(KD, 128, KFH, 128).transpose(2, 1, 0, 3).reshape(KFH, 128, KD * 128)
    )


def _pack_w2(W2e, half):
    sl = W2e[half * FH : (half + 1) * FH].astype(ml_dtypes.bfloat16)
    return np.ascontiguousarray(
        sl.reshape(KFH, 128, KD, 128).transpose(2, 1, 0, 3).reshape(KD, 128, KFH * 128)
    )


def _pack_b1(b1e, half):
    sl = b1e[half * FH : (half + 1) * FH]
    return np.ascontiguousarray(sl.reshape(KFH, 128).T.astype(np.float32))


def _pack_inputs(xf, idx_list, pairs, N1, N2, W1, b1, W2):
    xf_bf = xf.astype(ml_dtypes.bfloat16)
    xA = {}
    xB = {}
    for a, b in pairs:
        xA[a] = _pack_x(xf_bf, idx_list[a], N1)
        xB[b] = _pack_x(xf_bf, idx_list[b], N2)
    in_maps = []
    for a, b in pairs:
        for half in range(2):
            in_maps.append(
                {
                    "xA": xA[a],
                    "xB": xB[b],
                    "w1A": _pack_w1(W1[a], half),
                    "w1B": _pack_w1(W1[b], half),
                    "w2A": _pack_w2(W2[a], half),
                    "w2B": _pack_w2(W2[b], half),
                    "b1A": _pack_b1(b1[a], half),
                    "b1B": _pack_b1(b1[b], half),
                }
            )
    return in_maps


def _combine(results, idx_list, gate_list, pairs, N1, N2, T, b2):
    out = np.zeros((T, D), dtype=np.float32)
    for i, (a, b) in enumerate(pairs):
        r0, r1 = results[2 * i], results[2 * i + 1]
        for e, name, N in ((a, "yA", N1), (b, "yB", N2)):
            tok = idx_list[e]
            if len(tok) == 0:
                continue
            y = (r0[name] + r1[name]).transpose(2, 0, 1).reshape(N, D)[: len(tok)]
            g = gate_list[e][:, None]
            out[tok] += g * (y + b2[e][None, :])
    return out


def kernel(x, W1, b1, W2, b2, Wg, bg, top_k):
    x = np.asarray(x)
    W1 = np.asarray(W1, dtype=np.float32)
    b1 = np.asarray(b1, dtype=np.float32)
    W2 = np.asarray(W2, dtype=np.float32)
    b2 = np.asarray(b2, dtype=np.float32)
    Wg = np.asarray(Wg, dtype=np.float32)
    bg = np.asarray(bg, dtype=np.float32)
    top_k = int(np.asarray(top_k))

    B, S, Din = x.shape
    xf, idx_list, gate_list, pairs, N1, N2 = _plan(x, Wg, bg, top_k)
    nc = _get_kernel(N1, N2)
    in_maps = _pack_inputs(xf, idx_list, pairs, N1, N2, W1, b1, W2)
    res = run_bass_kernel_spmd(nc, in_maps, list(range(E)))
    out = _combine(res.results, idx_list, gate_list, pairs, N1, N2, B * S, b2)
    return out.reshape(B, S, Din).astype(np.float32)


# revision 46
# speedup vs baseline: 1.0286x; 1.0041x over previous
"""MoE layer (top-2 routing, 8 experts) on 8 Trainium2 NeuronCores.

Strategy (expert parallelism + 2-way F-sharding for load balance):
  - Host computes the gate (logits -> top-k -> softmax) and routes tokens
    (the host-side equivalent of the all-to-all).
  - Experts are paired (largest token count with smallest); each pair is
    F-sharded across two cores: core 2i holds columns [0, F/2) of experts
    (big_i, small_i), core 2i+1 holds columns [F/2, F).  Each core computes
    partial y sums for ALL tokens of both its experts; the host adds the
    two halves.  This flattens the per-core token count from max_e(count_e)
    to (max big + max small)/2, and keeps all weights SBUF-resident:
      W1 halves 2x2MB + W2 halves 2x2MB per expert -> 16MB/core in SBUF.
  - Per (expert-half, token-chunk) the FFN runs transpose-free:
      mm1:  h^T[f,c] = sum_k W1_blk[k,f].T @ x^T[k,c]   (W1 SBUF-resident)
      gelu: ACT engine, exact (erf) Gelu, bias b1 fused
      mm2:  y^T[d,c] = sum_f W2_blk[f,d].T @ h^T[f,c]   (W2 SBUF-resident)
    Weights/activations bf16 (full PE rate), fp32 PSUM accumulation.
    Weight residency means the PE never waits on weight DMA after the
    first f-tile, so the tensor engine stays at full clock (no HAM
    re-throttle) for the whole kernel.
  - b2 is applied on the host during the combine (y partials exclude it).

Hardcoded problem shape: x [4, 2048, 1024], E=8 experts, D=1024, F=4096.
"""

import numpy as np
import ml_dtypes

import concourse.bass as bass
import concourse.mybir as mybir
import concourse.tile as tile
from concourse import bacc
from concourse.bass_utils import run_bass_kernel_spmd

D = 1024
F = 4096
E = 8
KD = D // 128    # 8 k-tiles over D
FH = F // 2      # F half per shard
KFH = FH // 128  # 16 f-tiles per shard
NT = 512         # max token chunk width (PSUM bank = 512 fp32)

_KERNEL_CACHE = {}


def _chunks(N, first=None):
    """Token chunks (multiples of 8, each <= NT, each >= ~240).

    Any width >= ~240 runs at full PE rate (the ~97ns LDWEIGHTS hides
    behind the previous matmul's streaming); `first` carves off a small
    leading chunk so the kernel head only waits on a small x transfer.
    """
    out, c0 = [], 0
    if first is not None and N >= first + 240:
        out.append((0, first))
        c0 = first
        N -= first
    nch = -(-N // NT)
    base = (N // nch) & ~7
    rem = N - base * nch
    assert rem % 8 == 0
    widths = [base + 8 * (i < rem // 8) for i in range(nch)]
    for w in widths:
        out.append((c0, w))
        c0 += w
    return out


def _build_kernel(N1: int, N2: int):
    """Per-core kernel: two F-half expert shards, weights SBUF-resident.

    Shard A processes N1 tokens, shard B processes N2 tokens (both
    multiples of 8).  Uniform across all 8 cores (SPMD).
    """
    bf16 = mybir.dt.bfloat16
    f32 = mybir.dt.float32

    nc = bacc.Bacc("TRN2", target_bir_lowering=False, debug=False, num_devices=8)

    xd = [
        nc.dram_tensor("xA", [128, KD, N1], bf16, kind="ExternalInput"),
        nc.dram_tensor("xB", [128, KD, N2], bf16, kind="ExternalInput"),
    ]
    w1d = [
        nc.dram_tensor("w1A", [KFH, 128, KD * 128], bf16, kind="ExternalInput"),
        nc.dram_tensor("w1B", [KFH, 128, KD * 128], bf16, kind="ExternalInput"),
    ]
    w2d = [
        nc.dram_tensor("w2A", [KD, 128, KFH * 128], bf16, kind="ExternalInput"),
        nc.dram_tensor("w2B", [KD, 128, KFH * 128], bf16, kind="ExternalInput"),
    ]
    b1d = [
        nc.dram_tensor("b1A", [128, KFH], f32, kind="ExternalInput"),
        nc.dram_tensor("b1B", [128, KFH], f32, kind="ExternalInput"),
    ]
    yd = [
        nc.dram_tensor("yA", [KD, 128, N1], f32, kind="ExternalOutput"),
        nc.dram_tensor("yB", [KD, 128, N2], f32, kind="ExternalOutput"),
    ]
    warm = nc.dram_tensor("warm", [128, 8], f32, kind="ExternalOutput")

    # Flat segment list: (shard s, chunk offset, width)
    segs = [(0, c0, w) for c0, w in _chunks(N1)] + [(1, c0, w) for c0, w in _chunks(N2)]

    with tile.TileContext(nc) as tc:
        with (
            tc.tile_pool(name="const", bufs=1) as const,
            tc.tile_pool(name="w1r", bufs=2 * KFH) as w1r,
            tc.tile_pool(name="w2r", bufs=2 * KD) as w2r,
            tc.tile_pool(name="xp", bufs=3) as xp,
            tc.tile_pool(name="hp", bufs=KFH + 1) as hp,
            tc.tile_pool(name="yp", bufs=4) as yp,
            tc.tile_pool(name="psA", bufs=4, space="PSUM") as psA,
            tc.tile_pool(name="psB", bufs=4, space="PSUM") as psB,
        ):
            # PE warmup: dummy matmuls on zeroed scratch keep the tensor
            # engine busy (ramping to the full HAM p-state) while the first
            # real weight/x DMAs land; sized so the real stream starts fully
            # warm and never stalls after that (a stall costs ~3us of
            # half-clock re-ramp).  The tiny result is stored to a scratch
            # output so nothing gets dead-code-eliminated.  Memsets lead the
            # gpsimd queue so the warmup starts ~6.5us.
            w_s = const.tile([128, 128], bf16, tag="warm_w")
            x_s = const.tile([128, NT], bf16, tag="warm_x")
            nc.gpsimd.memset(w_s[:], 0)
            nc.gpsimd.memset(x_s[:], 0)
            ps_w = psB.tile([128, NT], f32, tag="ps2")
            NWARM = 15
            for i in range(NWARM):
                nc.tensor.matmul(
                    ps_w[:], w_s[:], x_s[:], start=(i == 0), stop=(i == NWARM - 1)
                )
            warm_sb = const.tile([128, 8], f32, tag="warm_y")
            nc.vector.tensor_scalar_add(warm_sb[:], ps_w[:, :8], 0.0)

            # x prefetch ring: one tile per segment chunk, 3 deep, on the
            # scalar queue (HW DGE: setup pipelines with transfers, unlike
            # the gpsimd SWDGE path which costs ~1us serial per dma_start).
            # Chunk 0's eight per-k DMAs serialize ~1us each, so they split
            # across BOTH queues (k0-3 gpsimd, k4-7 scalar): each queue
            # carries half the serialized depth and the last tile lands
            # ~2us earlier; worst case matches the single-queue path.
            x_tiles = {}

            def prefetch_x(si):
                s, c0, w = segs[si]
                t = xp.tile([128, KD, NT], bf16, tag="x")
                if si == 0:
                    for k in range(KD):
                        eng = nc.gpsimd if k < KD // 2 else nc.scalar
                        eng.dma_start(t[:, k, :w], xd[s][:, k, c0 : c0 + w])
                else:
                    nc.scalar.dma_start(t[:, :, :w], xd[s][:, :, c0 : c0 + w])
                x_tiles[si] = t

            prefetch_x(0)
            prefetch_x(1)

            # biases behind chunk 0's x on the gpsimd queue (tiny; needed
            # by the first gelu, ~1.4us after the real stream starts)
            b1_sb = []
            for s in range(2):
                t = const.tile([128, KFH], f32, tag=f"b1_{s}")
                nc.gpsimd.dma_start(t[:], b1d[s][:])
                b1_sb.append(t)
            # warm store parks at the back of the gpsimd queue
            nc.gpsimd.dma_start(warm[:], warm_sb[:])

            # resident weights on the sync queue, in consumption order:
            # w1A, w2A, w1B, w2B (per-tile DMAs so the first matmul only
            # waits on its own 256KB slice)
            w1_sb = [[], []]
            w2_sb = [[], []]
            for s in range(2):
                for fi in range(KFH):
                    t = w1r.tile([128, KD * 128], bf16, tag="w1")
                    nc.sync.dma_start(t[:], w1d[s][fi])
                    w1_sb[s].append(t)
                for d in range(KD):
                    t = w2r.tile([128, KFH * 128], bf16, tag="w2")
                    nc.sync.dma_start(t[:], w2d[s][d])
                    w2_sb[s].append(t)

            for si, (s, c0, w) in enumerate(segs):
                if si + 2 < len(segs):
                    prefetch_x(si + 2)

                h_t = []
                for fi in range(KFH):
                    ps = psA.tile([128, NT], f32)
                    for k in range(KD):
                        nc.tensor.matmul(
                            ps[:, :w],
                            w1_sb[s][fi][:, k * 128 : (k + 1) * 128],
                            x_tiles[si][:, k, :w],
                            start=(k == 0),
                            stop=(k == KD - 1),
                        )
                    ht = hp.tile([128, NT], bf16, tag="h")
                    nc.scalar.activation(
                        ht[:, :w],
                        ps[:, :w],
                        mybir.ActivationFunctionType.Gelu,
                        bias=b1_sb[s][:, fi : fi + 1],
                    )
                    h_t.append(ht)

                for d in range(KD):
                    ps2 = psB.tile([128, NT], f32, tag="ps2")
                    for k2 in range(KFH):
                        nc.tensor.matmul(
                            ps2[:, :w],
                            w2_sb[s][d][:, k2 * 128 : (k2 + 1) * 128],
                            h_t[k2][:, :w],
                            start=(k2 == 0),
                            stop=(k2 == KFH - 1),
                        )
                    y_t = yp.tile([128, NT], f32)
                    nc.vector.tensor_scalar_add(y_t[:, :w], ps2[:, :w], 0.0)
                    # sync queue (HW DGE): the gpsimd SWDGE drain costs ~5us
                    # at kernel tail, the sync drain doesn't
                    nc.sync.dma_start(yd[s][d, :, c0 : c0 + w], y_t[:, :w])

    nc.compile()
    return nc


def _get_kernel(N1: int, N2: int):
    key = (N1, N2)
    if key not in _KERNEL_CACHE:
        _KERNEL_CACHE[key] = _build_kernel(N1, N2)
    return _KERNEL_CACHE[key]


def _route(xf, Wg, bg, top_k):
    """Replicate the reference gate: logits -> top-k -> softmax."""
    logits = xf.astype(np.float32) @ Wg.astype(np.float32) + bg.astype(np.float32)
    # jax.lax.top_k: values sorted descending, ties broken by lower index.
    order = np.argsort(-logits, axis=1, kind="stable")
    sel = order[:, :top_k]                                      # [T, K]
    vals = np.take_along_axis(logits, sel, axis=1)              # [T, K]
    vmax = vals.max(axis=1, keepdims=True)
    ex = np.exp((vals - vmax).astype(np.float32))
    w = ex / ex.sum(axis=1, keepdims=True)                      # [T, K]
    return sel, w.astype(np.float32)


def _plan(x, Wg, bg, top_k):
    """Routing plan: per-expert token indices/gates + big/small pairing."""
    B, S, _ = x.shape
    xf = np.ascontiguousarray(x.reshape(B * S, D).astype(np.float32))
    sel, w = _route(xf, Wg, bg, top_k)
    idx_list, gate_list = [], []
    for e in range(E):
        hit = (sel == e)                    # [T, K]
        tok = np.nonzero(hit.any(axis=1))[0]
        kslot = hit[tok].argmax(axis=1)
        idx_list.append(tok)
        gate_list.append(w[tok, kslot])
    order = sorted(range(E), key=lambda e: -len(idx_list[e]))
    bigs = order[:4]                # 4 largest, descending count
    smalls = order[4:][::-1]        # 4 smallest, ascending count
    pairs = list(zip(bigs, smalls))  # (largest, smallest), ...
    pad = lambda n: max(128, -(-n // 8) * 8)
    N1 = pad(max(len(idx_list[a]) for a, _ in pairs))
    N2 = pad(max(len(idx_list[b]) for _, b in pairs))
    return xf, idx_list, gate_list, pairs, N1, N2


def _pack_x(xf_bf, tok, N):
    xe = np.zeros((N, D), dtype=ml_dtypes.bfloat16)
    xe[: len(tok)] = xf_bf[tok]
    return np.ascontiguousarray(xe.reshape(N, KD, 128).transpose(2, 1, 0))


def _pack_w1(W1e, half):
    sl = W1e[:, half * FH : (half + 1) * FH].astype(ml_dtypes.bfloat16)
    return np.ascontiguousarray(
        sl.reshape(KD, 128, KFH, 128).transpose(2, 1, 0, 3).reshape(KFH, 128, KD * 128)
    )


def _pack_w2(W2e, half):
    sl = W2e[half * FH : (half + 1) * FH].astype(ml_dtypes.bfloat16)
    return np.ascontiguousarray(
        sl.reshape(KFH, 128, KD, 128).transpose(2, 1, 0, 3).reshape(KD, 128, KFH * 128)
    )


def _pack_b1(b1e, half):
    sl = b1e[half * FH : (half + 1) * FH]
    return np.ascontiguousarray(sl.reshape(KFH, 128).T.astype(np.float32))


def _pack_inputs(xf, idx_list, pairs, N1, N2, W1, b1, W2):
    xf_bf = xf.astype(ml_dtypes.bfloat16)
    xA = {}
    xB = {}
    for a, b in pairs:
        xA[a] = _pack_x(xf_bf, idx_list[a], N1)
        xB[b] = _pack_x(xf_bf, idx_list[b], N2)
    in_maps = []
    for a, b in pairs:
        for half in range(2):
            in_maps.append(
                {
                    "xA": xA[a],
                    "xB": xB[b],
                    "w1A": _pack_w1(W1[a], half),
                    "w1B": _pack_w1(W1[b], half),
                    "w2A": _pack_w2(W2[a], half),
                    "w2B": _pack_w2(W2[b], half),
                    "b1A": _pack_b1(b1[a], half),
                    "b1B": _pack_b1(b1[b], half),
                }
            )
    return in_maps


def _combine(results, idx_list, gate_list, pairs, N1, N2, T, b2):
    out = np.zeros((T, D), dtype=np.float32)
    for i, (a, b) in enumerate(pairs):
        r0, r1 = results[2 * i], results[2 * i + 1]
        for e, name, N in ((a, "yA", N1), (b, "yB", N2)):
            tok = idx_list[e]
            if len(tok) == 0:
                continue
            y = (r0[name] + r1[name]).transpose(2, 0, 1).reshape(N, D)[: len(tok)]
            g = gate_list[e][:, None]
            out[tok] += g * (y + b2[e][None, :])
    return out


def kernel(x, W1, b1, W2, b2, Wg, bg, top_k):
    x = np.asarray(x)
    W1 = np.asarray(W1, dtype=np.float32)
    b1 = np.asarray(b1, dtype=np.float32)
    W2 = np.asarray(W2, dtype=np.float32)
    b2 = np.asarray(b2, dtype=np.float32)
    Wg = np.asarray(Wg, dtype=np.float32)
    bg = np.asarray(bg, dtype=np.float32)
    top_k = int(np.asarray(top_k))

    B, S, Din = x.shape
    xf, idx_list, gate_list, pairs, N1, N2 = _plan(x, Wg, bg, top_k)
    nc = _get_kernel(N1, N2)
    in_maps = _pack_inputs(xf, idx_list, pairs, N1, N2, W1, b1, W2)
    res = run_bass_kernel_spmd(nc, in_maps, list(range(E)))
    out = _combine(res.results, idx_list, gate_list, pairs, N1, N2, B * S, b2)
    return out.reshape(B, S, Din).astype(np.float32)
